# revision 2
# baseline (speedup 1.0000x reference)
"""Trainium2 Bass kernel for CompressDCT (blockwise 8x8 2D DCT + quantize).

Reference computation (encoder, the graded path):
    X = einsum('ij,ncpjqk,lk->ncpiql', D, x_blocks, D)   # D @ block @ D.T
    X = clip(round(X / q_table), -128, 127)              # q_table == ones
Decoder path (is_encoder == 0):
    out = D.T @ (block * q_table) @ D

Strategy: pure data parallel over 8 NeuronCores; each core processes 128
of the 1024 (N*C) 256x256 images.

Per-core kernel, per [128, 256] chunk (half of one image), with
K = kron(I_16, D) block-diagonal [128,128] and C = K.T (encoder):
    MM1: Yt = matmul(lhsT=chunk_tile, rhs=[C|C])  -> (K@chunk).T twice
    copy useful halves PSUM -> SBUF (DVE, rounds to float32r)
    MM2: Z  = matmul(lhsT=Yt,         rhs=[C|C])  -> (K@chunk) @ K.T twice
    quantize: ACT copy fp32 -> int8 (hardware round-half-even + saturate
              to [-128,127], exactly matching round + clip)
    DMA out int8 on the ACT HW-DGE queue (input DMAs ride the SP queue)

All matmuls run in float32r: with the moving operand 256 wide the PE
streams 1 column/cycle (vs 4 cycles/col for plain fp32), so the constant
is streamed duplicated [C|C] and only the useful 128-col halves of each
256-wide PSUM result are copied out. float32r keeps ~13-14 mantissa bits;
measured flip rate of the rounded int8 output vs fp64 is ~1.3e-4
(rel err ~1.3e-2, inside the 2e-2 gate). No cross-core communication.
"""
import os
import sys

import numpy as np

try:
    import concourse.bass as bass  # noqa: F401
except ImportError:
    sys.path.insert(0, "/opt/trn_rl_repo")

import concourse.bacc as bacc
import concourse.tile as tile
from concourse import mybir
from concourse.bass_utils import run_bass_kernel_spmd

BLOCK = 8
N_CORES = 8
# Full input: (16, 64, 256, 256) fp32. Shard along N: 2 N x 64 C = 128 images/core.
IMGS_PER_CORE = 128
H = W = 256

_CACHE = {}
LAST_RESULTS = None  # BassKernelResults of the most recent run (for profiling)
TRACE = False


def _dct_mat():
    # Identical arithmetic to the reference's _dct_mat (fp64 -> fp32 cast).
    i = np.arange(BLOCK)
    k = np.arange(BLOCK)[:, None]
    D = np.cos(np.pi * (2 * i + 1) * k / (2 * BLOCK))
    s = np.full((BLOCK, 1), np.sqrt(2.0 / BLOCK))
    s[0, 0] = np.sqrt(1.0 / BLOCK)
    return (D * s).astype(np.float32)


def _build(encoder: bool, include_q: bool, repeat: int = 0):
    """float32r dup-constant kernel. repeat=0: straight-line (graded path);
    repeat>0: body wrapped in For_i(0, repeat) for differential timing."""
    from contextlib import ExitStack

    nc = bacc.Bacc("TRN2", target_bir_lowering=False, debug=False)
    dt = mybir.dt
    f32r = dt.float32r

    # x is declared float32r so the DMA (which never casts) can feed the
    # f32r matmuls directly; the PE truncates the extra mantissa bits.
    x_in = nc.dram_tensor(
        "x", [IMGS_PER_CORE, H, W], f32r, kind="ExternalInput"
    ).ap()
    c_in = nc.dram_tensor("kt", [128, 256], dt.float32, kind="ExternalInput").ap()
    if include_q:
        q_in = nc.dram_tensor("rq", [128, 256], dt.float32, kind="ExternalInput").ap()
    odt = dt.int8 if encoder else dt.float32
    out = nc.dram_tensor(
        "out", [IMGS_PER_CORE // 2, 128, 1024], odt, kind="ExternalOutput"
    ).ap()

    n_chunks = IMGS_PER_CORE * 2  # two [128, 256] row-halves per image

    with tile.TileContext(nc) as tc:
        with (
            tc.tile_pool(name="const", bufs=1) as cpool,
            tc.tile_pool(name="pin", bufs=6) as pin,
            tc.tile_pool(name="py", bufs=8) as py,
            tc.tile_pool(name="pout", bufs=6) as pout,
            tc.tile_pool(name="psy", bufs=3, space="PSUM") as psy,
            tc.tile_pool(name="psz", bufs=3, space="PSUM") as psz,
            ExitStack() as lp,
        ):
            # constant staged through fp32 then rounded on-device to f32r
            t_c_stage = cpool.tile([128, 256], dt.float32)
            nc.sync.dma_start(t_c_stage[:], c_in[:])
            t_c = cpool.tile([128, 256], f32r)
            nc.vector.tensor_copy(t_c[:], t_c_stage[:])
            if include_q:
                t_q = cpool.tile([128, 256], dt.float32)
                nc.sync.dma_start(t_q[:], q_in[:])

            if repeat:
                lp.enter_context(tc.For_i(0, repeat, 1))

            in_tiles = {}
            out_tiles = {}

            def stage1(chunk):
                img, half = divmod(chunk, 2)
                if half == 0:
                    t_in = pin.tile([128, 512], f32r, tag="t_in")
                    nc.sync.dma_start(
                        t_in[:].rearrange("p (h w) -> p h w", h=2),
                        x_in[img].rearrange("(h p) w -> p h w", h=2),
                    )
                    in_tiles[img] = t_in
                t_in = in_tiles[img]
                c0 = half * 256

                src = t_in
                s0 = c0
                if not encoder and include_q:
                    # decoder pre-multiplies blocks by q (rounds into f32r)
                    t_xq = pin.tile([128, 256], f32r, tag="xq")
                    nc.vector.tensor_mul(t_xq[:], t_in[:, c0 : c0 + 256], t_q[:])
                    src, s0 = t_xq, 0

                p_y = psy.tile([128, 512], dt.float32, tag="p_y")
                nc.tensor.matmul(
                    p_y[:, 0:256], src[:, s0 : s0 + 128], t_c[:],
                    start=True, stop=True,
                )
                nc.tensor.matmul(
                    p_y[:, 256:512], src[:, s0 + 128 : s0 + 256], t_c[:],
                    start=True, stop=True,
                )
                t_y = py.tile([128, 256], f32r, tag="t_y")
                # the useful 128-col halves sit at cols 0:128 and 256:384
                nc.vector.tensor_copy(
                    t_y[:].rearrange("p (t x) -> p t x", t=2),
                    p_y[:].rearrange("p (t x) -> p t x", t=2)[:, :, 0:128],
                )
                return t_y

            def stage2(chunk, t_y):
                pair, quarter = divmod(chunk, 4)

                p_z = psz.tile([128, 512], dt.float32, tag="p_z")
                nc.tensor.matmul(
                    p_z[:, 0:256], t_y[:, 0:128], t_c[:], start=True, stop=True
                )
                nc.tensor.matmul(
                    p_z[:, 256:512], t_y[:, 128:256], t_c[:], start=True, stop=True
                )

                if quarter == 0:
                    out_tiles[pair] = pout.tile(
                        [128, 1024], dt.int8 if encoder else dt.float32,
                        tag="t_o", name=f"t_o_{pair}"
                    )
                t_o = out_tiles[pair]
                q0 = quarter * 256

                if encoder and include_q:
                    # X / q, then round+clip via the int8 cast
                    t_m = py.tile([128, 256], dt.float32, tag="m")
                    nc.vector.tensor_mul(
                        t_m[:].rearrange("p (t x) -> p t x", t=2),
                        p_z[:].rearrange("p (t x) -> p t x", t=2)[:, :, 0:128],
                        t_q[:].rearrange("p (t x) -> p t x", t=2),
                    )
                    nc.scalar.copy(t_o[:, q0 : q0 + 256], t_m[:])
                else:
                    nc.scalar.copy(
                        t_o[:, q0 : q0 + 256].rearrange("p (t x) -> p t x", t=2),
                        p_z[:].rearrange("p (t x) -> p t x", t=2)[:, :, 0:128],
                    )

                if quarter == 3:
                    # output DMAs ride the ACT HW-DGE queue; inputs use SP
                    nc.scalar.dma_start(out[pair], t_o[:])
                    del out_tiles[pair]

            for chunk in range(n_chunks):
                stage2(chunk, stage1(chunk))

    nc.compile()
    return nc


def _get(encoder: bool, include_q: bool):
    key = (encoder, include_q)
    if key not in _CACHE:
        _CACHE[key] = _build(encoder, include_q)
    return _CACHE[key]


def kernel(x, q_table, is_encoder):
    global LAST_RESULTS
    x = np.ascontiguousarray(np.asarray(x, dtype=np.float32))
    q = np.asarray(q_table, dtype=np.float32)
    enc = bool(int(np.asarray(is_encoder)))
    include_q = not np.all(q == 1.0)

    N, C, H_, W_ = x.shape
    assert (H_, W_) == (H, W) and N * C == N_CORES * IMGS_PER_CORE

    D = _dct_mat()
    K = np.kron(np.eye(16, dtype=np.float32), D)  # [128, 128] block-diagonal
    c_one = (K.T if enc else K).astype(np.float32)
    const = np.ascontiguousarray(np.concatenate([c_one, c_one], axis=1))

    shards = x.reshape(N_CORES, IMGS_PER_CORE, H, W)
    in_maps = []
    for c in range(N_CORES):
        m = {"x": shards[c], "kt": const}
        if include_q:
            qt = np.tile(q, (16, 32)).astype(np.float32)  # [128, 256]
            m["rq"] = np.ascontiguousarray(1.0 / qt if enc else qt)
        in_maps.append(m)

    nc = _get(enc, include_q)
    res = run_bass_kernel_spmd(
        nc, in_maps, list(range(N_CORES)),
        trace=TRACE or bool(os.environ.get("KERNEL_TRACE")),
    )
    LAST_RESULTS = res

    o = np.stack([res.results[c]["out"] for c in range(N_CORES)])
    # [core, pair, p, (img_in_pair, half, w)] -> [core, pair, img_in_pair, half, p, w]
    o = o.reshape(N_CORES, IMGS_PER_CORE // 2, 128, 2, 2, 256)
    o = o.transpose(0, 1, 3, 4, 2, 5)
    return np.ascontiguousarray(o).reshape(N, C, H, W).astype(np.float32)


# revision 4
# speedup vs baseline: 1.3500x; 1.3500x over previous
"""Trainium2 Bass kernel for CompressDCT (blockwise 8x8 2D DCT + quantize).

Reference computation (encoder, the graded path):
    X = einsum('ij,ncpjqk,lk->ncpiql', D, x_blocks, D)   # D @ block @ D.T
    X = clip(round(X / q_table), -128, 127)              # q_table == ones
Decoder path (is_encoder == 0):
    out = D.T @ (block * q_table) @ D

Strategy: pure data parallel over 8 NeuronCores; each core processes 128
of the 1024 (N*C) 256x256 images.

Per-core kernel, per [128, 256] chunk (half of one image), with
K = kron(I_16, D) block-diagonal [128,128] and C = K.T (encoder):
    MM1: Yt = matmul(lhsT=chunk_tile, rhs=[C|C])  -> (K@chunk).T twice
    copy useful halves PSUM -> SBUF (DVE, rounds to float32r)
    MM2: Z  = matmul(lhsT=Yt,         rhs=[C|C])  -> (K@chunk) @ K.T twice
    quantize: ACT copy fp32 -> int8 (hardware round-half-even + saturate
              to [-128,127], exactly matching round + clip)
    DMA out int8 on the ACT HW-DGE queue (input DMAs ride the SP queue)

All matmuls run in float32r: with the moving operand 256 wide the PE
streams 1 column/cycle (vs 4 cycles/col for plain fp32), so the constant
is streamed duplicated [C|C] and only the useful 128-col halves of each
256-wide PSUM result are copied out. float32r keeps ~13-14 mantissa bits;
measured flip rate of the rounded int8 output vs fp64 is ~1.3e-4
(rel err ~1.3e-2, inside the 2e-2 gate). No cross-core communication.
"""
import os
import sys

import numpy as np

try:
    import concourse.bass as bass  # noqa: F401
except ImportError:
    sys.path.insert(0, "/opt/trn_rl_repo")

import concourse.bacc as bacc
import concourse.tile as tile
from concourse import mybir
from concourse.bass_utils import run_bass_kernel_spmd

BLOCK = 8
N_CORES = 8
# Full input: (16, 64, 256, 256) fp32. Shard along N: 2 N x 64 C = 128 images/core.
IMGS_PER_CORE = 128
H = W = 256

_CACHE = {}
LAST_RESULTS = None  # BassKernelResults of the most recent run (for profiling)
TRACE = False


def _dct_mat():
    # Identical arithmetic to the reference's _dct_mat (fp64 -> fp32 cast).
    i = np.arange(BLOCK)
    k = np.arange(BLOCK)[:, None]
    D = np.cos(np.pi * (2 * i + 1) * k / (2 * BLOCK))
    s = np.full((BLOCK, 1), np.sqrt(2.0 / BLOCK))
    s[0, 0] = np.sqrt(1.0 / BLOCK)
    return (D * s).astype(np.float32)


def _build(encoder: bool, include_q: bool, repeat: int = 0):
    """float32r dup-constant kernel. repeat=0: straight-line (graded path);
    repeat>0: body wrapped in For_i(0, repeat) for differential timing."""
    from contextlib import ExitStack

    nc = bacc.Bacc("TRN2", target_bir_lowering=False, debug=False)
    dt = mybir.dt
    f32r = dt.float32r

    # x is declared float32r so the DMA (which never casts) can feed the
    # f32r matmuls directly; the PE truncates the extra mantissa bits.
    x_in = nc.dram_tensor(
        "x", [IMGS_PER_CORE, H, W], f32r, kind="ExternalInput"
    ).ap()
    c_in = nc.dram_tensor("kt", [128, 256], dt.float32, kind="ExternalInput").ap()
    if include_q:
        q_in = nc.dram_tensor("rq", [128, 256], dt.float32, kind="ExternalInput").ap()
    odt = dt.int8 if encoder else dt.float32
    out = nc.dram_tensor(
        "out", [IMGS_PER_CORE // 2, 128, 1024], odt, kind="ExternalOutput"
    ).ap()

    n_chunks = IMGS_PER_CORE * 2  # two [128, 256] row-halves per image

    with tile.TileContext(nc) as tc:
        with (
            tc.tile_pool(name="const", bufs=1) as cpool,
            tc.tile_pool(name="pin", bufs=6) as pin,
            tc.tile_pool(name="py", bufs=8) as py,
            tc.tile_pool(name="pout", bufs=6) as pout,
            tc.tile_pool(name="psy", bufs=3, space="PSUM") as psy,
            tc.tile_pool(name="psz", bufs=3, space="PSUM") as psz,
            ExitStack() as lp,
        ):
            # constant staged through fp32 then rounded on-device to f32r
            t_c_stage = cpool.tile([128, 256], dt.float32)
            nc.sync.dma_start(t_c_stage[:], c_in[:])
            t_c = cpool.tile([128, 256], f32r)
            nc.vector.tensor_copy(t_c[:], t_c_stage[:])
            if include_q:
                t_q = cpool.tile([128, 256], dt.float32)
                nc.sync.dma_start(t_q[:], q_in[:])

            if repeat:
                lp.enter_context(tc.For_i(0, repeat, 1))

            in_tiles = {}
            out_tiles = {}

            def stage1(chunk):
                img, half = divmod(chunk, 2)
                if half == 0:
                    t_in = pin.tile([128, 512], f32r, tag="t_in")
                    nc.sync.dma_start(
                        t_in[:].rearrange("p (h w) -> p h w", h=2),
                        x_in[img].rearrange("(h p) w -> p h w", h=2),
                    )
                    in_tiles[img] = t_in
                t_in = in_tiles[img]
                c0 = half * 256

                src = t_in
                s0 = c0
                if not encoder and include_q:
                    # decoder pre-multiplies blocks by q (rounds into f32r)
                    t_xq = pin.tile([128, 256], f32r, tag="xq")
                    nc.vector.tensor_mul(t_xq[:], t_in[:, c0 : c0 + 256], t_q[:])
                    src, s0 = t_xq, 0

                p_y = psy.tile([128, 512], dt.float32, tag="p_y")
                nc.tensor.matmul(
                    p_y[:, 0:256], src[:, s0 : s0 + 128], t_c[:],
                    start=True, stop=True,
                )
                nc.tensor.matmul(
                    p_y[:, 256:512], src[:, s0 + 128 : s0 + 256], t_c[:],
                    start=True, stop=True,
                )
                t_y = py.tile([128, 256], f32r, tag="t_y")
                # the useful 128-col halves sit at cols 0:128 and 256:384
                nc.vector.tensor_copy(
                    t_y[:].rearrange("p (t x) -> p t x", t=2),
                    p_y[:].rearrange("p (t x) -> p t x", t=2)[:, :, 0:128],
                )
                return t_y

            def stage2(chunk, t_y):
                pair, quarter = divmod(chunk, 4)

                p_z = psz.tile([128, 512], dt.float32, tag="p_z")
                nc.tensor.matmul(
                    p_z[:, 0:256], t_y[:, 0:128], t_c[:], start=True, stop=True
                )
                nc.tensor.matmul(
                    p_z[:, 256:512], t_y[:, 128:256], t_c[:], start=True, stop=True
                )

                if quarter == 0:
                    out_tiles[pair] = pout.tile(
                        [128, 1024], dt.int8 if encoder else dt.float32,
                        tag="t_o", name=f"t_o_{pair}"
                    )
                t_o = out_tiles[pair]
                q0 = quarter * 256

                if encoder and include_q:
                    # X / q, then round+clip via the int8 cast
                    t_m = py.tile([128, 256], dt.float32, tag="m")
                    nc.vector.tensor_mul(
                        t_m[:].rearrange("p (t x) -> p t x", t=2),
                        p_z[:].rearrange("p (t x) -> p t x", t=2)[:, :, 0:128],
                        t_q[:].rearrange("p (t x) -> p t x", t=2),
                    )
                    nc.scalar.copy(t_o[:, q0 : q0 + 256], t_m[:])
                else:
                    nc.scalar.copy(
                        t_o[:, q0 : q0 + 256].rearrange("p (t x) -> p t x", t=2),
                        p_z[:].rearrange("p (t x) -> p t x", t=2)[:, :, 0:128],
                    )

                if quarter == 3:
                    # output DMAs ride the ACT HW-DGE queue; inputs use SP
                    nc.scalar.dma_start(out[pair], t_o[:])
                    del out_tiles[pair]

            for chunk in range(n_chunks):
                stage2(chunk, stage1(chunk))

    nc.compile()
    return nc


def _get(encoder: bool, include_q: bool):
    key = (encoder, include_q)
    if key not in _CACHE:
        _CACHE[key] = _build(encoder, include_q)
    return _CACHE[key]


def kernel(x, q_table, is_encoder):
    global LAST_RESULTS
    x = np.ascontiguousarray(np.asarray(x, dtype=np.float32))
    q = np.asarray(q_table, dtype=np.float32)
    enc = bool(int(np.asarray(is_encoder)))
    include_q = not np.all(q == 1.0)

    N, C, H_, W_ = x.shape
    assert (H_, W_) == (H, W) and N * C == N_CORES * IMGS_PER_CORE

    D = _dct_mat()
    K = np.kron(np.eye(16, dtype=np.float32), D)  # [128, 128] block-diagonal
    c_one = (K.T if enc else K).astype(np.float32)
    const = np.ascontiguousarray(np.concatenate([c_one, c_one], axis=1))

    shards = x.reshape(N_CORES, IMGS_PER_CORE, H, W)
    in_maps = []
    for c in range(N_CORES):
        m = {"x": shards[c], "kt": const}
        if include_q:
            qt = np.tile(q, (16, 32)).astype(np.float32)  # [128, 256]
            m["rq"] = np.ascontiguousarray(1.0 / qt if enc else qt)
        in_maps.append(m)

    nc = _get(enc, include_q)
    res = run_bass_kernel_spmd(
        nc, in_maps, list(range(N_CORES)),
        trace=TRACE or bool(os.environ.get("KERNEL_TRACE")),
    )
    LAST_RESULTS = res

    o = np.stack([res.results[c]["out"] for c in range(N_CORES)])
    # [core, pair, p, (img_in_pair, half, w)] -> [core, pair, img_in_pair, half, p, w]
    o = o.reshape(N_CORES, IMGS_PER_CORE // 2, 128, 2, 2, 256)
    o = o.transpose(0, 1, 3, 4, 2, 5)
    return np.ascontiguousarray(o).reshape(N, C, H, W).astype(np.float32)
